# revision 16
# baseline (speedup 1.0000x reference)
"""CharRNN Trainium2 kernel (data-parallel over batch across 8 NeuronCores).

Reference computation (per batch row b, seq len L=1024):
    xp_t   = emb[x[b,t]] @ Wx + b_ih          (Wx = W_ih[:64])
    h_t    = tanh(xp_t + h_{t-1} @ Wh)        (Wh = W_ih[64:])
    logits = h_t @ W_ho + b_ho

Device strategy (per core, batch shard of 32 rows):
  - Feature-major ("transposed") layout: hidden state lives as h^T tiles
    [128 partitions = hidden-dim half, free = batch], so the recurrent
    matmuls use the full 128-wide PE array with Wh quadrants stationary.
  - Embedding gather + input projection fused into a one-hot matmul:
    xp^T = A'^T @ onehot, where A' = [emb @ Wx ; b_ih] (97 x 256) and
    onehot[v, col] = (x[col] == v), built on-chip from a broadcast matmul
    of the index row and a DVE is_equal against an iota column.
  - The batch shard is split into G=2 independent groups of 16 so the
    per-step tanh (ACT) of one group pipelines against the PE matmuls of
    the other group; separate PSUM banks per group avoid bank-conflict
    serialization.
  - L is processed in 64 chunks of 16 steps; per chunk the input
    projections land in PSUM via 2 matmuls/group, the 16 sequential steps
    accumulate h@Wh on top, and the output projection streams the chunk's
    h buffer through W_ho into the logits output.
"""

import os
import numpy as np

import concourse.bacc as bacc
import concourse.mybir as mybir
import concourse.tile as tile
from concourse import bass_utils
from concourse.bass import ds, ts

dt = mybir.dt

B, L, V, E, H = 256, 1024, 96, 64, 256
NCORES = 8
BS = B // NCORES          # 32 batch rows per core
G = 2                     # pipeline groups per core
GB = BS // G              # 16 rows per group
LC = 16                   # timesteps per chunk
NCH = L // LC             # 64 chunks

# dtype knobs. All-f32 keeps absmax rel-err ~1e-6 vs the reference.
# Speed/accuracy trade (measured in sim + numpy): EMB_DT=bf16 -> ~1.6e-3,
# +LOGIT_DT=bf16 -> ~2.9e-3, +SCAN_DT=bf16 -> ~5.4e-3 rel err, each step
# shaving PE time (bf16 also halves LDWEIGHTS cost on the recurrent matmuls).
SCAN_DT = dt.float32      # Wh / h storage (recurrence)
EMB_DT = dt.float32       # A' + onehot (one-hot selection: only quantizes A)
LOGIT_DT = dt.float32     # W_ho + h as seen by the output projection
F32 = dt.float32
BF16 = dt.bfloat16

_NP = {dt.float32: np.float32, dt.bfloat16: None}  # bf16 handled via view

_build_cache = {}
DEBUG = False


def _np_cast(a, d):
    if d in (dt.float32, dt.float32r):
        return np.ascontiguousarray(a, dtype=np.float32)
    if d == dt.bfloat16:
        import ml_dtypes
        return np.ascontiguousarray(a.astype(ml_dtypes.bfloat16))
    raise ValueError(d)


def _build():
    key = (SCAN_DT, EMB_DT, LOGIT_DT)
    if key in _build_cache:
        return _build_cache[key]

    nc = bacc.Bacc(None, target_bir_lowering=False)

    # index broadcast path is exact in bf16 (integer values < 256)
    xb_d = nc.declare_dram_parameter("xb", (NCH, G * LC * GB), BF16, isOutput=False)
    ap_d = nc.declare_dram_parameter("ap", (V + 1, 2, 128), EMB_DT, isOutput=False)
    iota_d = nc.declare_dram_parameter("iota", (V, 1), F32, isOutput=False)
    ones_d = nc.declare_dram_parameter("ones", (1, V), BF16, isOutput=False)
    wh_d = nc.declare_dram_parameter("wh", (128, 2, 2, 128), SCAN_DT, isOutput=False)
    who_d = nc.declare_dram_parameter("who", (128, 2, V), LOGIT_DT, isOutput=False)
    bho_d = nc.declare_dram_parameter("bho", (V, 1), F32, isOutput=False)
    hin_d = nc.declare_dram_parameter("hin", (128, 2, G, GB), SCAN_DT, isOutput=False)
    lout_d = nc.declare_dram_parameter("lout", (NCH, V, G, LC, GB), F32, isOutput=True)
    hout_d = nc.declare_dram_parameter("hout", (128, 2, G, GB), F32, isOutput=True)
    if DEBUG:
        ohdbg_d = nc.declare_dram_parameter("ohdbg", (V + 1, G, LC, GB), F32, isOutput=True)
        xpdbg_d = nc.declare_dram_parameter("xpdbg", (G, 128, 2, LC, GB), F32, isOutput=True)
        hbdbg_d = nc.declare_dram_parameter("hbdbg", (G, 128, 2, LC, GB), F32, isOutput=True)

    Tanh = mybir.ActivationFunctionType.Tanh

    with tile.TileContext(nc) as tc:
        with (
            tc.tile_pool(name="const", bufs=1) as cp,
            tc.tile_pool(name="xrow", bufs=3) as xrp,
            tc.tile_pool(name="oh", bufs=2) as ohp,
            tc.tile_pool(name="hb", bufs=2) as hbp,
            tc.tile_pool(name="lsb", bufs=2) as lsbp,
            tc.tile_pool(name="pxp", bufs=2, space="PSUM") as pxp,
            tc.tile_pool(name="poh", bufs=2, space="PSUM") as pohp,
            tc.tile_pool(name="plg", bufs=2, space="PSUM") as plgp,
        ):
            # constants / weights, loaded once
            a_sb = cp.tile([V + 1, 2, 128], EMB_DT)
            iota_sb = cp.tile([V, 1], F32)
            ones_sb = cp.tile([1, V], BF16)
            wh_sb = cp.tile([128, 2, 2, 128], SCAN_DT)
            who_sb = cp.tile([128, 2, V], LOGIT_DT)
            bho_sb = cp.tile([V, 1], F32)
            hin_sb = cp.tile([128, 2, G, GB], SCAN_DT)
            nc.sync.dma_start(a_sb[:], ap_d[:])
            nc.sync.dma_start(iota_sb[:], iota_d[:])
            nc.sync.dma_start(ones_sb[:], ones_d[:])
            nc.sync.dma_start(wh_sb[:], wh_d[:])
            nc.sync.dma_start(who_sb[:], who_d[:])
            nc.sync.dma_start(bho_sb[:], bho_d[:])
            nc.sync.dma_start(hin_sb[:], hin_d[:])

            prev_hb = None
            for c in range(NCH):
                # ---- chunk prologue: onehot + input projections ----
                xrow = xrp.tile([1, G * LC * GB], BF16, tag="xrow")
                nc.sync.dma_start(xrow[:], xb_d[ds(c, 1), :])

                oh_ps = pohp.tile([V, G, LC, GB], F32, tag="ohps")
                nc.tensor.matmul(oh_ps[:], ones_sb[:], xrow[:], start=True, stop=True)

                oh = ohp.tile([V + 1, G, LC, GB], EMB_DT, tag="oh")
                nc.vector.tensor_scalar(
                    oh[0:V], oh_ps[:], iota_sb[:], None, mybir.AluOpType.is_equal
                )
                nc.gpsimd.memset(oh[V : V + 1], 1.0)

                xpg = [
                    pxp.tile([128, 2, LC, GB], F32, tag=f"xp{g}", name=f"xp{g}_{c}")
                    for g in range(G)
                ]
                for g in range(G):
                    for ih in range(2):
                        # First matmul into this PSUM bank: start=True marks the
                        # whole 2KB zero-region pending-zero; everything after
                        # accumulates (first write per element = overwrite).
                        nc.tensor.matmul(
                            xpg[g][:, ih],
                            a_sb[:, ih],
                            oh[:, g],
                            start=(ih == 0),
                            stop=False,
                            skip_group_check=True,
                        )

                if DEBUG and c == 0:
                    ohf = ohp.tile([V + 1, G, LC, GB], F32, tag="ohf", name="ohf")
                    nc.vector.tensor_copy(ohf[:], oh[:])
                    nc.sync.dma_start(ohdbg_d[:], ohf[:])
                    for g in range(G):
                        xpf = ohp.tile([128, 2, LC, GB], F32, tag="xpf", name=f"xpf{g}")
                        nc.vector.tensor_copy(xpf[:], xpg[g][:])
                        nc.sync.dma_start(xpdbg_d[g], xpf[:])

                # ---- sequential scan (16 steps, quadrant-major over groups) ----
                hb = [
                    hbp.tile([128, 2, LC, GB], SCAN_DT, tag=f"hb{g}", name=f"hb{g}_{c}")
                    for g in range(G)
                ]
                for t in range(LC):
                    for ih in range(2):
                        for jh in range(2):
                            for g in range(G):
                                if t == 0:
                                    if c == 0:
                                        rhs = hin_sb[:, jh, g]
                                    else:
                                        rhs = prev_hb[g][:, jh, LC - 1]
                                else:
                                    rhs = hb[g][:, jh, t - 1]
                                nc.tensor.matmul(
                                    xpg[g][:, ih, t],
                                    wh_sb[:, jh, ih],
                                    rhs,
                                    start=False,
                                    stop=(t == LC - 1 and ih == 1 and jh == 1),
                                    skip_group_check=True,
                                )
                    for g in range(G):
                        nc.scalar.activation(hb[g][:, :, t], xpg[g][:, :, t], Tanh)

                if DEBUG and c == 0:
                    for g in range(G):
                        nc.sync.dma_start(hbdbg_d[g], hb[g][:])

                # ---- chunk epilogue: output projection ----
                lg_ps = plgp.tile([V, G, LC, GB], F32, tag="lgps")
                for g in range(G):
                    if LOGIT_DT == SCAN_DT:
                        hsrc = hb[g]
                    else:
                        hsrc = lsbp.tile([128, 2, LC, GB], LOGIT_DT, tag=f"hbb{g}")
                        nc.vector.tensor_copy(hsrc[:], hb[g][:])
                    for ih in range(2):
                        nc.tensor.matmul(
                            lg_ps[:, g],
                            who_sb[:, ih],
                            hsrc[:, ih],
                            start=(ih == 0),
                            stop=(ih == 1),
                        )
                lsb = lsbp.tile([V, G, LC, GB], F32, tag="lsb")
                nc.vector.tensor_scalar_add(lsb[:], lg_ps[:], bho_sb[:])
                nc.sync.dma_start(lout_d[c], lsb[:])

                if c == NCH - 1:
                    if SCAN_DT == F32:
                        for g in range(G):
                            nc.sync.dma_start(
                                hout_d[:, :, g, :], hb[g][:, :, LC - 1]
                            )
                    else:
                        hfin = lsbp.tile([128, 2, G, GB], F32, tag="hfin")
                        for g in range(G):
                            nc.vector.tensor_copy(
                                hfin[:, :, g, :], hb[g][:, :, LC - 1]
                            )
                        nc.sync.dma_start(hout_d[:], hfin[:])

                prev_hb = hb

    nc.compile()
    _build_cache[key] = nc
    return nc


def _pack_inputs(x, hidden, emb, W_ih, b_ih, W_ho, b_ho):
    """Host-side packing into per-core input maps."""
    x = np.asarray(x)
    hidden = np.asarray(hidden, dtype=np.float32)
    emb = np.asarray(emb, dtype=np.float32)
    W_ih = np.asarray(W_ih, dtype=np.float32)
    b_ih = np.asarray(b_ih, dtype=np.float32)
    W_ho = np.asarray(W_ho, dtype=np.float32)
    b_ho = np.asarray(b_ho, dtype=np.float32)

    Wx, Wh = W_ih[:E], W_ih[E:]
    A = emb @ Wx                                   # [V, H] fp32
    Ap = np.concatenate([A, b_ih[None, :]], 0)     # [V+1, H]
    ap_p = _np_cast(Ap.reshape(V + 1, 2, 128), EMB_DT)
    iota_p = np.arange(V, dtype=np.float32).reshape(V, 1)
    ones_p = _np_cast(np.ones((1, V), np.float32), BF16)
    wh_p = _np_cast(
        Wh.reshape(2, 128, 2, 128).transpose(1, 0, 2, 3), SCAN_DT
    )  # [j, jh, ih, i]
    who_p = _np_cast(W_ho.reshape(2, 128, V).transpose(1, 0, 2), LOGIT_DT)
    bho_p = b_ho.reshape(V, 1)

    in_maps = []
    for core in range(NCORES):
        xs = x[core * BS : (core + 1) * BS]        # [32, 1024] int
        hs = hidden[core * BS : (core + 1) * BS]   # [32, 256]
        # xb[c, g, t, b] = xs[g*GB+b, c*LC+t]
        xb = (
            xs.reshape(G, GB, NCH, LC)
            .transpose(2, 0, 3, 1)
            .reshape(NCH, G * LC * GB)
            .astype(np.float32)
        )
        # hin[p, half, g, b] = hs[g*GB+b, half*128+p]
        hin = hs.reshape(G, GB, 2, 128).transpose(3, 2, 0, 1)
        in_maps.append(
            {
                "xb": _np_cast(xb, BF16),
                "ap": ap_p,
                "iota": iota_p,
                "ones": ones_p,
                "wh": wh_p,
                "who": who_p,
                "bho": bho_p,
                "hin": _np_cast(hin, SCAN_DT),
            }
        )
    return in_maps


def _unpack_outputs(results):
    logits = np.empty((B, L, V), np.float32)
    hidden = np.empty((B, H), np.float32)
    for core in range(NCORES):
        lout = results[core]["lout"].reshape(NCH, V, G, LC, GB)
        # logits[g*GB+b, c*LC+t, v] = lout[c, v, g, t, b]
        logits[core * BS : (core + 1) * BS] = lout.transpose(2, 4, 0, 3, 1).reshape(
            BS, L, V
        )
        hout = results[core]["hout"].reshape(128, 2, G, GB)
        hidden[core * BS : (core + 1) * BS] = hout.transpose(2, 3, 1, 0).reshape(BS, H)
    return logits, hidden


def kernel(x, hidden, emb, W_ih, b_ih, W_ho, b_ho, _collect=None):
    nc = _build()
    in_maps = _pack_inputs(x, hidden, emb, W_ih, b_ih, W_ho, b_ho)
    kwargs = {}
    if _collect is not None:
        kwargs = _collect.pop("kwargs", {})
    res = bass_utils.run_bass_kernel_spmd(
        nc, in_maps, core_ids=list(range(NCORES)), **kwargs
    )
    if _collect is not None:
        _collect["res"] = res
    return _unpack_outputs(res.results)


# revision 22
# speedup vs baseline: 1.2383x; 1.2383x over previous
"""CharRNN Trainium2 kernel (data-parallel over batch across 8 NeuronCores).

Reference computation (per batch row b, seq len L=1024):
    xp_t   = emb[x[b,t]] @ Wx + b_ih          (Wx = W_ih[:64])
    h_t    = tanh(xp_t + h_{t-1} @ Wh)        (Wh = W_ih[64:])
    logits = h_t @ W_ho + b_ho

Device strategy (per core, batch shard of 32 rows):
  - Feature-major ("transposed") layout: hidden state lives as h^T tiles
    [128 partitions = hidden-dim half, free = batch], so the recurrent
    matmuls use the full 128-wide PE array with Wh quadrants stationary.
  - Embedding gather + input projection fused into a one-hot matmul:
    xp^T = A'^T @ onehot, where A' = [emb @ Wx ; b_ih] (97 x 256) and
    onehot[v, col] = (x[col] == v), built on-chip from a broadcast matmul
    of the index row and a DVE is_equal against an iota column.
  - The batch shard is split into G=2 independent groups of 16 so the
    per-step tanh (ACT) of one group pipelines against the PE matmuls of
    the other group; separate PSUM banks per group avoid bank-conflict
    serialization.
  - L is processed in 64 chunks of 16 steps; per chunk the input
    projections land in PSUM via 2 matmuls/group, the 16 sequential steps
    accumulate h@Wh on top, and the output projection streams the chunk's
    h buffer through W_ho into the logits output.
"""

import os
import numpy as np

import concourse.bacc as bacc
import concourse.mybir as mybir
import concourse.tile as tile
from concourse import bass_utils
from concourse.bass import ds, ts

dt = mybir.dt

B, L, V, E, H = 256, 1024, 96, 64, 256
NCORES = 8
BS = B // NCORES          # 32 batch rows per core
G = 2                     # pipeline groups per core
GB = BS // G              # 16 rows per group
LC = 16                   # timesteps per chunk
NCH = L // LC             # 64 chunks

# dtype knobs. Measured absmax rel-err vs the reference on hardware:
#   all-f32: 9.2e-7; EMB/LOGIT f32r (default): 1.9e-4.
# f32r runs the N>=256 projection matmuls at 1 cycle/row (vs 4 for f32) with
# ~14-bit multiply precision. The recurrence stays f32: f32r is no faster at
# N=16 and its rounding would compound over the 1024 sequential steps.
# bf16 options (not default): EMB bf16 -> ~1.6e-3, +LOGIT bf16 -> ~2.9e-3,
# +SCAN bf16 -> ~5.4e-3 rel err; SCAN bf16 also halves LDWEIGHTS cost.
SCAN_DT = dt.float32      # Wh / h storage (recurrence)
EMB_DT = dt.float32r      # A' + onehot (one-hot selection: only quantizes A)
LOGIT_DT = dt.float32r    # W_ho + h as seen by the output projection
F32 = dt.float32
BF16 = dt.bfloat16

_NP = {dt.float32: np.float32, dt.bfloat16: None}  # bf16 handled via view

_build_cache = {}
DEBUG = False


def _np_cast(a, d):
    if d in (dt.float32, dt.float32r):
        return np.ascontiguousarray(a, dtype=np.float32)
    if d == dt.bfloat16:
        import ml_dtypes
        return np.ascontiguousarray(a.astype(ml_dtypes.bfloat16))
    raise ValueError(d)


def _build():
    key = (SCAN_DT, EMB_DT, LOGIT_DT)
    if key in _build_cache:
        return _build_cache[key]

    nc = bacc.Bacc(None, target_bir_lowering=False)

    # index broadcast path is exact in bf16 (integer values < 256)
    xb_d = nc.declare_dram_parameter("xb", (NCH, G * LC * GB), BF16, isOutput=False)
    ap_d = nc.declare_dram_parameter("ap", (V + 1, 2, 128), EMB_DT, isOutput=False)
    iota_d = nc.declare_dram_parameter("iota", (V, 1), F32, isOutput=False)
    ones_d = nc.declare_dram_parameter("ones", (1, V), BF16, isOutput=False)
    onesrow_d = nc.declare_dram_parameter(
        "onesrow", (1, G * LC * GB), EMB_DT, isOutput=False
    )
    wh_d = nc.declare_dram_parameter("wh", (128, 2, 2, 128), SCAN_DT, isOutput=False)
    who_d = nc.declare_dram_parameter("who", (128, 2, V), LOGIT_DT, isOutput=False)
    bho_d = nc.declare_dram_parameter("bho", (V, 1), F32, isOutput=False)
    hin_d = nc.declare_dram_parameter("hin", (128, 2, G, GB), SCAN_DT, isOutput=False)
    lout_d = nc.declare_dram_parameter("lout", (NCH, V, G, LC, GB), F32, isOutput=True)
    hout_d = nc.declare_dram_parameter("hout", (128, 2, G, GB), F32, isOutput=True)
    if DEBUG:
        ohdbg_d = nc.declare_dram_parameter("ohdbg", (V + 1, G, LC, GB), F32, isOutput=True)
        xpdbg_d = nc.declare_dram_parameter("xpdbg", (G, 128, 2, LC, GB), F32, isOutput=True)
        hbdbg_d = nc.declare_dram_parameter("hbdbg", (G, 128, 2, LC, GB), F32, isOutput=True)

    Tanh = mybir.ActivationFunctionType.Tanh

    with tile.TileContext(nc) as tc:
        with (
            tc.tile_pool(name="const", bufs=1) as cp,
            tc.tile_pool(name="xrow", bufs=3) as xrp,
            tc.tile_pool(name="oh", bufs=2) as ohp,
            tc.tile_pool(name="hb", bufs=2) as hbp,
            tc.tile_pool(name="lsb", bufs=2) as lsbp,
            tc.tile_pool(name="pxp", bufs=2, space="PSUM") as pxp,
            tc.tile_pool(name="poh", bufs=2, space="PSUM") as pohp,
            tc.tile_pool(name="plg", bufs=2, space="PSUM") as plgp,
        ):
            # constants / weights, loaded once
            a_sb = cp.tile([V + 1, 2, 128], EMB_DT)
            iota_sb = cp.tile([V, 1], F32)
            ones_sb = cp.tile([1, V], BF16)
            wh_sb = cp.tile([128, 2, 2, 128], SCAN_DT)
            who_sb = cp.tile([128, 2, V], LOGIT_DT)
            bho_sb = cp.tile([V, 1], F32)
            hin_sb = cp.tile([128, 2, G, GB], SCAN_DT)
            nc.sync.dma_start(a_sb[:], ap_d[:])
            nc.sync.dma_start(iota_sb[:], iota_d[:])
            nc.sync.dma_start(ones_sb[:], ones_d[:])
            nc.sync.dma_start(wh_sb[:], wh_d[:])
            nc.sync.dma_start(who_sb[:], who_d[:])
            nc.sync.dma_start(bho_sb[:], bho_d[:])
            nc.sync.dma_start(hin_sb[:], hin_d[:])

            prev_hb = None
            for c in range(NCH):
                # ---- chunk prologue: onehot + input projections ----
                xrow = xrp.tile([1, G * LC * GB], BF16, tag="xrow")
                nc.sync.dma_start(xrow[:], xb_d[ds(c, 1), :])

                oh_ps = pohp.tile([V, G, LC, GB], F32, tag="ohps")
                nc.tensor.matmul(oh_ps[:], ones_sb[:], xrow[:], start=True, stop=True)

                oh = ohp.tile([V + 1, G, LC, GB], EMB_DT, tag="oh")
                nc.vector.tensor_scalar(
                    oh[0:V], oh_ps[:], iota_sb[:], None, mybir.AluOpType.is_equal
                )
                nc.sync.dma_start(oh[V : V + 1], onesrow_d[:])

                xpg = [
                    pxp.tile([128, 2, LC, GB], F32, tag=f"xp{g}", name=f"xp{g}_{c}")
                    for g in range(G)
                ]
                for g in range(G):
                    for ih in range(2):
                        # First matmul into this PSUM bank: start=True marks the
                        # whole 2KB zero-region pending-zero; everything after
                        # accumulates (first write per element = overwrite).
                        nc.tensor.matmul(
                            xpg[g][:, ih],
                            a_sb[:, ih],
                            oh[:, g],
                            start=(ih == 0),
                            stop=False,
                            skip_group_check=True,
                        )

                if DEBUG and c == 0:
                    ohf = ohp.tile([V + 1, G, LC, GB], F32, tag="ohf", name="ohf")
                    nc.vector.tensor_copy(ohf[:], oh[:])
                    nc.sync.dma_start(ohdbg_d[:], ohf[:])
                    for g in range(G):
                        xpf = ohp.tile([128, 2, LC, GB], F32, tag="xpf", name=f"xpf{g}")
                        nc.vector.tensor_copy(xpf[:], xpg[g][:])
                        nc.sync.dma_start(xpdbg_d[g], xpf[:])

                # ---- sequential scan (16 steps, quadrant-major over groups) ----
                hb = [
                    hbp.tile([128, 2, LC, GB], SCAN_DT, tag=f"hb{g}", name=f"hb{g}_{c}")
                    for g in range(G)
                ]
                for t in range(LC):
                    for ih in range(2):
                        for jh in range(2):
                            for g in range(G):
                                if t == 0:
                                    if c == 0:
                                        rhs = hin_sb[:, jh, g]
                                    else:
                                        rhs = prev_hb[g][:, jh, LC - 1]
                                else:
                                    rhs = hb[g][:, jh, t - 1]
                                nc.tensor.matmul(
                                    xpg[g][:, ih, t],
                                    wh_sb[:, jh, ih],
                                    rhs,
                                    start=False,
                                    stop=(t == LC - 1 and ih == 1 and jh == 1),
                                    skip_group_check=True,
                                )
                    for g in range(G):
                        nc.scalar.activation(hb[g][:, :, t], xpg[g][:, :, t], Tanh)

                if DEBUG and c == 0:
                    for g in range(G):
                        nc.sync.dma_start(hbdbg_d[g], hb[g][:])

                # ---- chunk epilogue: output projection ----
                lg_ps = plgp.tile([V, G, LC, GB], F32, tag="lgps")
                for g in range(G):
                    if LOGIT_DT == SCAN_DT:
                        hsrc = hb[g]
                    else:
                        # walrus requires f32r matmul operands to be written
                        # as f32r (producer rounds), so an explicit convert
                        # copy is needed; DVE has plenty of headroom here.
                        hsrc = lsbp.tile([128, 2, LC, GB], LOGIT_DT, tag=f"hbb{g}")
                        nc.vector.tensor_copy(hsrc[:], hb[g][:])
                    for ih in range(2):
                        nc.tensor.matmul(
                            lg_ps[:, g],
                            who_sb[:, ih],
                            hsrc[:, ih],
                            start=(ih == 0),
                            stop=(ih == 1),
                        )
                lsb = lsbp.tile([V, G, LC, GB], F32, tag="lsb")
                nc.vector.tensor_scalar_add(lsb[:], lg_ps[:], bho_sb[:])
                nc.sync.dma_start(lout_d[c], lsb[:])

                if c == NCH - 1:
                    if SCAN_DT == F32:
                        for g in range(G):
                            nc.sync.dma_start(
                                hout_d[:, :, g, :], hb[g][:, :, LC - 1]
                            )
                    else:
                        hfin = lsbp.tile([128, 2, G, GB], F32, tag="hfin")
                        for g in range(G):
                            nc.vector.tensor_copy(
                                hfin[:, :, g, :], hb[g][:, :, LC - 1]
                            )
                        nc.sync.dma_start(hout_d[:], hfin[:])

                prev_hb = hb

    nc.compile()
    _build_cache[key] = nc
    return nc


def _pack_inputs(x, hidden, emb, W_ih, b_ih, W_ho, b_ho):
    """Host-side packing into per-core input maps."""
    x = np.asarray(x)
    hidden = np.asarray(hidden, dtype=np.float32)
    emb = np.asarray(emb, dtype=np.float32)
    W_ih = np.asarray(W_ih, dtype=np.float32)
    b_ih = np.asarray(b_ih, dtype=np.float32)
    W_ho = np.asarray(W_ho, dtype=np.float32)
    b_ho = np.asarray(b_ho, dtype=np.float32)

    Wx, Wh = W_ih[:E], W_ih[E:]
    A = emb @ Wx                                   # [V, H] fp32
    Ap = np.concatenate([A, b_ih[None, :]], 0)     # [V+1, H]
    ap_p = _np_cast(Ap.reshape(V + 1, 2, 128), EMB_DT)
    iota_p = np.arange(V, dtype=np.float32).reshape(V, 1)
    ones_p = _np_cast(np.ones((1, V), np.float32), BF16)
    wh_p = _np_cast(
        Wh.reshape(2, 128, 2, 128).transpose(1, 0, 2, 3), SCAN_DT
    )  # [j, jh, ih, i]
    who_p = _np_cast(W_ho.reshape(2, 128, V).transpose(1, 0, 2), LOGIT_DT)
    bho_p = b_ho.reshape(V, 1)

    in_maps = []
    for core in range(NCORES):
        xs = x[core * BS : (core + 1) * BS]        # [32, 1024] int
        hs = hidden[core * BS : (core + 1) * BS]   # [32, 256]
        # xb[c, g, t, b] = xs[g*GB+b, c*LC+t]
        xb = (
            xs.reshape(G, GB, NCH, LC)
            .transpose(2, 0, 3, 1)
            .reshape(NCH, G * LC * GB)
            .astype(np.float32)
        )
        # hin[p, half, g, b] = hs[g*GB+b, half*128+p]
        hin = hs.reshape(G, GB, 2, 128).transpose(3, 2, 0, 1)
        in_maps.append(
            {
                "xb": _np_cast(xb, BF16),
                "ap": ap_p,
                "iota": iota_p,
                "ones": ones_p,
                "onesrow": _np_cast(
                    np.ones((1, G * LC * GB), np.float32), EMB_DT
                ),
                "wh": wh_p,
                "who": who_p,
                "bho": bho_p,
                "hin": _np_cast(hin, SCAN_DT),
            }
        )
    return in_maps


def _unpack_outputs(results):
    logits = np.empty((B, L, V), np.float32)
    hidden = np.empty((B, H), np.float32)
    for core in range(NCORES):
        lout = results[core]["lout"].reshape(NCH, V, G, LC, GB)
        # logits[g*GB+b, c*LC+t, v] = lout[c, v, g, t, b]
        logits[core * BS : (core + 1) * BS] = lout.transpose(2, 4, 0, 3, 1).reshape(
            BS, L, V
        )
        hout = results[core]["hout"].reshape(128, 2, G, GB)
        hidden[core * BS : (core + 1) * BS] = hout.transpose(2, 3, 1, 0).reshape(BS, H)
    return logits, hidden


def kernel(x, hidden, emb, W_ih, b_ih, W_ho, b_ho, _collect=None):
    nc = _build()
    in_maps = _pack_inputs(x, hidden, emb, W_ih, b_ih, W_ho, b_ho)
    kwargs = {}
    if _collect is not None:
        kwargs = _collect.pop("kwargs", {})
    res = bass_utils.run_bass_kernel_spmd(
        nc, in_maps, core_ids=list(range(NCORES)), **kwargs
    )
    if _collect is not None:
        _collect["res"] = res
    return _unpack_outputs(res.results)


# revision 34
# speedup vs baseline: 1.3237x; 1.0690x over previous
"""CharRNN Trainium2 kernel (data-parallel over batch across 8 NeuronCores).

Reference computation (per batch row b, seq len L=1024):
    xp_t   = emb[x[b,t]] @ Wx + b_ih          (Wx = W_ih[:64])
    h_t    = tanh(xp_t + h_{t-1} @ Wh)        (Wh = W_ih[64:])
    logits = h_t @ W_ho + b_ho

Device strategy (per core, batch shard of 32 rows):
  - Feature-major ("transposed") layout: hidden state lives as h^T tiles
    [128 partitions = hidden-dim half, free = batch], so the recurrent
    matmuls use the full 128-wide PE array with Wh quadrants stationary.
  - Embedding gather + input projection fused into a one-hot matmul:
    xp^T = A'^T @ onehot, where A' = [emb @ Wx ; b_ih] (97 x 256) and
    onehot[v, col] = (x[col] == v), built on-chip from a broadcast matmul
    of the index row and a DVE is_equal against an iota column.
  - The batch shard is split into G=2 independent groups of 16 so the
    per-step tanh (ACT) of one group pipelines against the PE matmuls of
    the other group; separate PSUM banks per group avoid bank-conflict
    serialization.
  - L is processed in 64 chunks of 16 steps; per chunk the input
    projections land in PSUM via 2 matmuls/group, the 16 sequential steps
    accumulate h@Wh on top, and the output projection streams the chunk's
    h buffer through W_ho into the logits output.
"""

import os
import numpy as np

import concourse.bacc as bacc
import concourse.mybir as mybir
import concourse.tile as tile
from concourse import bass_utils
from concourse.bass import ds, ts

dt = mybir.dt

B, L, V, E, H = 256, 1024, 96, 64, 256
NCORES = 8
BS = B // NCORES          # 32 batch rows per core
G = 2                     # pipeline groups per core
GB = BS // G              # 16 rows per group
LC = 16                   # timesteps per chunk
NCH = L // LC             # 64 chunks

# dtype knobs. Measured absmax rel-err vs the reference on hardware:
#   all-f32: 9.2e-7; EMB/LOGIT f32r (default): 1.9e-4.
# f32r runs the N>=256 projection matmuls at 1 cycle/row (vs 4 for f32) with
# ~14-bit multiply precision. The recurrence stays f32: f32r is no faster at
# N=16 and its rounding would compound over the 1024 sequential steps.
# bf16 options (not default): EMB bf16 -> ~1.6e-3, +LOGIT bf16 -> ~2.9e-3,
# +SCAN bf16 -> ~5.4e-3 rel err; SCAN bf16 also halves LDWEIGHTS cost.
SCAN_DT = dt.float32      # Wh / h storage (recurrence)
EMB_DT = dt.float32r      # A' + onehot (one-hot selection: only quantizes A)
LOGIT_DT = dt.float32r    # W_ho + h as seen by the output projection
F32 = dt.float32
BF16 = dt.bfloat16

_NP = {dt.float32: np.float32, dt.bfloat16: None}  # bf16 handled via view

_build_cache = {}
DEBUG = False


def _np_cast(a, d):
    if d in (dt.float32, dt.float32r):
        return np.ascontiguousarray(a, dtype=np.float32)
    if d == dt.bfloat16:
        import ml_dtypes
        return np.ascontiguousarray(a.astype(ml_dtypes.bfloat16))
    raise ValueError(d)


def _build():
    key = (SCAN_DT, EMB_DT, LOGIT_DT)
    if key in _build_cache:
        return _build_cache[key]

    nc = bacc.Bacc(None, target_bir_lowering=False)

    # index broadcast path is exact in bf16 (integer values < 256)
    xb_d = nc.declare_dram_parameter("xb", (NCH, G * LC * GB), BF16, isOutput=False)
    ap_d = nc.declare_dram_parameter("ap", (V + 1, 2, 128), EMB_DT, isOutput=False)
    iota_d = nc.declare_dram_parameter("iota", (V, 1), F32, isOutput=False)
    ones_d = nc.declare_dram_parameter("ones", (1, V), BF16, isOutput=False)
    onesrow_d = nc.declare_dram_parameter(
        "onesrow", (1, G * LC * GB), EMB_DT, isOutput=False
    )
    wh_d = nc.declare_dram_parameter("wh", (128, 2, 2, 128), SCAN_DT, isOutput=False)
    who_d = nc.declare_dram_parameter("who", (128, 2, V), LOGIT_DT, isOutput=False)
    bho_d = nc.declare_dram_parameter("bho", (V, 1), F32, isOutput=False)
    hin_d = nc.declare_dram_parameter("hin", (128, 2, G, GB), SCAN_DT, isOutput=False)
    lout_d = nc.declare_dram_parameter("lout", (NCH, V, G, LC, GB), F32, isOutput=True)
    hout_d = nc.declare_dram_parameter("hout", (128, 2, G, GB), F32, isOutput=True)
    if DEBUG:
        ohdbg_d = nc.declare_dram_parameter("ohdbg", (V + 1, G, LC, GB), F32, isOutput=True)
        xpdbg_d = nc.declare_dram_parameter("xpdbg", (G, 128, 2, LC, GB), F32, isOutput=True)
        hbdbg_d = nc.declare_dram_parameter("hbdbg", (G, 128, 2, LC, GB), F32, isOutput=True)

    Tanh = mybir.ActivationFunctionType.Tanh

    with tile.TileContext(nc) as tc:
        with (
            tc.tile_pool(name="const", bufs=1) as cp,
            tc.tile_pool(name="xrow", bufs=3) as xrp,
            tc.tile_pool(name="oh", bufs=2) as ohp,
            tc.tile_pool(name="hb", bufs=2) as hbp,
            tc.tile_pool(name="lsb", bufs=2) as lsbp,
            tc.tile_pool(name="pxp", bufs=2, space="PSUM") as pxp,
            tc.tile_pool(name="poh", bufs=2, space="PSUM") as pohp,
            tc.tile_pool(name="plg", bufs=2, space="PSUM") as plgp,
        ):
            # constants / weights, loaded once
            a_sb = cp.tile([V + 1, 2, 128], EMB_DT)
            iota_sb = cp.tile([V, 1], F32)
            ones_sb = cp.tile([1, V], BF16)
            wh_sb = cp.tile([128, 2, 2, 128], SCAN_DT)
            who_sb = cp.tile([128, 2, V], LOGIT_DT)
            bho_sb = cp.tile([V, 1], F32)
            hin_sb = cp.tile([128, 2, G, GB], SCAN_DT)
            nc.sync.dma_start(a_sb[:], ap_d[:])
            nc.sync.dma_start(iota_sb[:], iota_d[:])
            nc.sync.dma_start(ones_sb[:], ones_d[:])
            nc.sync.dma_start(wh_sb[:], wh_d[:])
            nc.sync.dma_start(who_sb[:], who_d[:])
            nc.sync.dma_start(bho_sb[:], bho_d[:])
            nc.sync.dma_start(hin_sb[:], hin_d[:])

            # Software-pipelined emission: the PE engine executes its queue
            # in order, so any not-yet-ready instruction at a chunk boundary
            # stalls the sequential scan. Prologue work for chunk c+1 (index
            # row DMA, broadcast, one-hot, input projections) and the logits
            # epilogue for chunk c-1 are emitted in the middle of chunk c's
            # scan, where their inputs have long been ready and the PE has
            # idle slots under the ACT-bound steady state.
            xrow_t, oh_t, xpg_t, hb_t = {}, {}, {}, {}

            def emit_xrow(c):
                if c >= NCH:
                    return
                xr = xrp.tile([1, G * LC * GB], BF16, tag="xrow", name=f"xrow{c}")
                nc.sync.dma_start(xr[:], xb_d[ds(c, 1), :])
                xrow_t[c] = xr

            def emit_onehot(c):
                if c >= NCH:
                    return
                oh_ps = pohp.tile([V, G, LC, GB], F32, tag="ohps", name=f"ohps{c}")
                nc.tensor.matmul(
                    oh_ps[:], ones_sb[:], xrow_t.pop(c)[:], start=True, stop=True
                )
                oh = ohp.tile([V + 1, G, LC, GB], EMB_DT, tag="oh", name=f"oh{c}")
                nc.vector.tensor_scalar(
                    oh[0:V], oh_ps[:], iota_sb[:], None, mybir.AluOpType.is_equal
                )
                nc.sync.dma_start(oh[V : V + 1], onesrow_d[:])
                oh_t[c] = oh

            def emit_amm(c, part):
                # one matmul per scan step so each PE insertion fits the
                # per-step idle under the ACT-bound steady state
                if c >= NCH:
                    return
                g, ih = divmod(part, 2)
                if part == 0:
                    xpg_t[c] = [
                        pxp.tile([128, 2, LC, GB], F32, tag=f"gg{g}", name=f"xp{g}_{c}")
                        for g in range(G)
                    ]
                xpg = xpg_t[c]
                oh = oh_t.pop(c) if part == 2 * G - 1 else oh_t[c]
                # First matmul into this PSUM bank: start=True marks the
                # whole 2KB zero-region pending-zero; everything after
                # accumulates (first write per element = overwrite).
                nc.tensor.matmul(
                    xpg[g][:, ih],
                    a_sb[:, ih],
                    oh[:, g],
                    start=(ih == 0),
                    stop=False,
                    skip_group_check=True,
                )

            lg_t, hsrc_t = {}, {}

            def emit_epilogue(c, part):
                # one output-projection matmul per scan step
                if c < 0:
                    return
                g, ih = divmod(part, 2)
                hb = hb_t[c]
                if part == 0:
                    lg_t[c] = plgp.tile([V, G, LC, GB], F32, tag="lgps", name=f"lgps{c}")
                lg_ps = lg_t[c]
                if LOGIT_DT == SCAN_DT:
                    hsrc = hb[g]
                elif ih == 0:
                    # walrus requires f32r matmul operands to be written
                    # as f32r (producer rounds), so an explicit convert
                    # copy is needed; DVE has plenty of headroom here.
                    hsrc = lsbp.tile(
                        [128, 2, LC, GB], LOGIT_DT, tag=f"hbb{g}", name=f"hbb{g}_{c}"
                    )
                    nc.vector.tensor_copy(hsrc[:], hb[g][:])
                    hsrc_t[(c, g)] = hsrc
                else:
                    hsrc = hsrc_t.pop((c, g))
                nc.tensor.matmul(
                    lg_ps[:, g],
                    who_sb[:, ih],
                    hsrc[:, ih],
                    start=(ih == 0),
                    stop=(ih == 1),
                )
                if part == 2 * G - 1:
                    if c != NCH - 1:
                        hb_t.pop(c)
                    lg_t.pop(c)
                    lsb = lsbp.tile([V, G, LC, GB], F32, tag="lsb", name=f"lsb{c}")
                    nc.vector.tensor_scalar_add(lsb[:], lg_ps[:], bho_sb[:])
                    nc.sync.dma_start(lout_d[c], lsb[:])

            # warm-up: chunk 0 fully, chunk 1's index row
            emit_xrow(0)
            emit_xrow(1)
            emit_onehot(0)
            for p in range(2 * G):
                emit_amm(0, p)

            if DEBUG:
                # debug taps read the chunk-0 psum input projections
                for g in range(G):
                    xpf = ohp.tile([128, 2, LC, GB], F32, tag="xpf", name=f"xpf{g}")
                    nc.vector.tensor_copy(xpf[:], xpg_t[0][g][:])
                    nc.sync.dma_start(xpdbg_d[g], xpf[:])

            for c in range(NCH):
                xpg = xpg_t.pop(c)
                hb = [
                    hbp.tile([128, 2, LC, GB], SCAN_DT, tag=f"hb{g}", name=f"hb{g}_{c}")
                    for g in range(G)
                ]
                hb_t[c] = hb
                prev_hb = hb_t.get(c - 1)
                for t in range(LC):
                    for ih in range(2):
                        for jh in range(2):
                            for g in range(G):
                                if t == 0:
                                    if c == 0:
                                        rhs = hin_sb[:, jh, g]
                                    else:
                                        rhs = prev_hb[g][:, jh, LC - 1]
                                else:
                                    rhs = hb[g][:, jh, t - 1]
                                nc.tensor.matmul(
                                    xpg[g][:, ih, t],
                                    wh_sb[:, jh, ih],
                                    rhs,
                                    start=False,
                                    stop=(t == LC - 1 and ih == 1 and jh == 1),
                                    skip_group_check=True,
                                )
                    for g in range(G):
                        nc.scalar.activation(hb[g][:, :, t], xpg[g][:, :, t], Tanh)
                    if t == 1:
                        emit_xrow(c + 2)
                        emit_onehot(c + 1)
                    elif 4 <= t < 4 + 2 * G:
                        emit_amm(c + 1, t - 4)
                    elif 9 <= t < 9 + 2 * G:
                        emit_epilogue(c - 1, t - 9)

                if DEBUG and c == 0:
                    for g in range(G):
                        nc.sync.dma_start(hbdbg_d[g], hb[g][:])

            for p in range(2 * G):
                emit_epilogue(NCH - 1, p)
            hb = hb_t[NCH - 1]
            if SCAN_DT == F32:
                for g in range(G):
                    nc.sync.dma_start(hout_d[:, :, g, :], hb[g][:, :, LC - 1])
            else:
                hfin = lsbp.tile([128, 2, G, GB], F32, tag="hfin")
                for g in range(G):
                    nc.vector.tensor_copy(hfin[:, :, g, :], hb[g][:, :, LC - 1])
                nc.sync.dma_start(hout_d[:], hfin[:])

    nc.compile()
    _build_cache[key] = nc
    return nc


def _pack_inputs(x, hidden, emb, W_ih, b_ih, W_ho, b_ho):
    """Host-side packing into per-core input maps."""
    x = np.asarray(x)
    hidden = np.asarray(hidden, dtype=np.float32)
    emb = np.asarray(emb, dtype=np.float32)
    W_ih = np.asarray(W_ih, dtype=np.float32)
    b_ih = np.asarray(b_ih, dtype=np.float32)
    W_ho = np.asarray(W_ho, dtype=np.float32)
    b_ho = np.asarray(b_ho, dtype=np.float32)

    Wx, Wh = W_ih[:E], W_ih[E:]
    A = emb @ Wx                                   # [V, H] fp32
    Ap = np.concatenate([A, b_ih[None, :]], 0)     # [V+1, H]
    ap_p = _np_cast(Ap.reshape(V + 1, 2, 128), EMB_DT)
    iota_p = np.arange(V, dtype=np.float32).reshape(V, 1)
    ones_p = _np_cast(np.ones((1, V), np.float32), BF16)
    wh_p = _np_cast(
        Wh.reshape(2, 128, 2, 128).transpose(1, 0, 2, 3), SCAN_DT
    )  # [j, jh, ih, i]
    who_p = _np_cast(W_ho.reshape(2, 128, V).transpose(1, 0, 2), LOGIT_DT)
    bho_p = b_ho.reshape(V, 1)

    in_maps = []
    for core in range(NCORES):
        xs = x[core * BS : (core + 1) * BS]        # [32, 1024] int
        hs = hidden[core * BS : (core + 1) * BS]   # [32, 256]
        # xb[c, g, t, b] = xs[g*GB+b, c*LC+t]
        xb = (
            xs.reshape(G, GB, NCH, LC)
            .transpose(2, 0, 3, 1)
            .reshape(NCH, G * LC * GB)
            .astype(np.float32)
        )
        # hin[p, half, g, b] = hs[g*GB+b, half*128+p]
        hin = hs.reshape(G, GB, 2, 128).transpose(3, 2, 0, 1)
        in_maps.append(
            {
                "xb": _np_cast(xb, BF16),
                "ap": ap_p,
                "iota": iota_p,
                "ones": ones_p,
                "onesrow": _np_cast(
                    np.ones((1, G * LC * GB), np.float32), EMB_DT
                ),
                "wh": wh_p,
                "who": who_p,
                "bho": bho_p,
                "hin": _np_cast(hin, SCAN_DT),
            }
        )
    return in_maps


def _unpack_outputs(results):
    logits = np.empty((B, L, V), np.float32)
    hidden = np.empty((B, H), np.float32)
    for core in range(NCORES):
        lout = results[core]["lout"].reshape(NCH, V, G, LC, GB)
        # logits[g*GB+b, c*LC+t, v] = lout[c, v, g, t, b]
        logits[core * BS : (core + 1) * BS] = lout.transpose(2, 4, 0, 3, 1).reshape(
            BS, L, V
        )
        hout = results[core]["hout"].reshape(128, 2, G, GB)
        hidden[core * BS : (core + 1) * BS] = hout.transpose(2, 3, 1, 0).reshape(BS, H)
    return logits, hidden


def kernel(x, hidden, emb, W_ih, b_ih, W_ho, b_ho, _collect=None):
    nc = _build()
    in_maps = _pack_inputs(x, hidden, emb, W_ih, b_ih, W_ho, b_ho)
    kwargs = {}
    if _collect is not None:
        kwargs = _collect.pop("kwargs", {})
    res = bass_utils.run_bass_kernel_spmd(
        nc, in_maps, core_ids=list(range(NCORES)), **kwargs
    )
    if _collect is not None:
        _collect["res"] = res
    return _unpack_outputs(res.results)
